# revision 14
# baseline (speedup 1.0000x reference)
"""Trainium2 Bass kernel for nn_EncoderLayer (pre-LN transformer encoder layer).

Reference computation (per batch element b):
    h  = LN1(x)
    h  = h + causal_attention(h)      # q=k=v=h, 8 heads, head dim 64
    h2 = LN2(h)
    out = h2 + relu(h2 @ W1 + b1) @ W2 + b2

Sharding: pure data-parallel over batch. B=64 is split 8 ways; each of the 8
NeuronCores runs the identical NEFF on its own 8-batch shard with the full
weights. No collectives.

Per-core layout strategy:
  - Activations live in [tokens(partition), D(free)] layout; LayerNorm reduces
    along the free dim with bn_stats/bn_aggr.
  - Matmuls contract along the partition dim, so attention scores and the FFN
    need a [D, tokens] transposed copy of the activations; that transpose is
    done with the DMA xbar (bf16, zero compute-engine cost).  The xbar writes
    256B-aligned output tiles, so the token axis of every transposed tensor is
    chunk-padded: [*, 4, 128] with tokens (c, j), j < 125 valid and the three
    pad columns zero (h_b rows 125-127 are memset).  Downstream matmul APs
    either span whole chunks (moving operands; pad columns produce garbage in
    pad output columns that nothing reads) or slice [0:125] (stationary
    operands).
  - Attention computes S^T = [keys, queries] so the exp (ScalarE) can read
    straight out of PSUM with the 1/sqrt(hd) scale folded in; causality is
    handled by restricting every matmul/exp to query chunks >= the key block
    plus one 125x125 upper-triangular mask multiply on the diagonal block.
    No max-subtraction is needed: post-LN logits are bounded (|s|/8 <~ 20) and
    the HW exp LUT is accurate there (probed 1.1e-5 rel err).
  - AV uses v as the 64-col stationary operand and E^T as the moving operand,
    with a col-packed ones-column matmul (tile_position=(0,64)) producing the
    softmax denominators Z concurrently.  A small PE transpose brings the
    attention output back to [tokens, D], where one scalar_tensor_tensor fuses
    the 1/Z normalize + residual add.
  - FFN: ff1^T = W1^T @ h2^T (W1 is naturally [D, dff] = the stationary
    layout), relu(.+b1) fused on ScalarE, then ff2 uses relu1^T as stationary
    to come back to [tokens, D] directly; residual + b2 fused on the way out.
  - All matmuls are bf16 (4x the fp32 rate on the PE); all accumulation and
    the residual spine stay fp32.
"""

import numpy as np

import concourse.bass as bass
import concourse.mybir as mybir
import concourse.tile as tile
from concourse import bacc
from concourse.bass_utils import run_bass_kernel_spmd

F32 = mybir.dt.float32
BF16 = mybir.dt.bfloat16
AF = mybir.ActivationFunctionType
OP = mybir.AluOpType

N_CORES = 8
B, S, D = 64, 500, 512
H, HD = 8, 64
FF = 2048
EPS = 1e-6

BPC = B // N_CORES  # batches per core
P = 125             # tokens per chunk
C = S // P          # 4 chunks per batch
CW = C * 128        # chunk-padded token width (512)
DC = D // 128       # 4 chunks of D
FC = FF // 128      # 16 chunks of dff


def build(n_batches=BPC, dbg=False):
    nc = bacc.Bacc(
        "TRN2",
        target_bir_lowering=False,
        debug=False,
        enable_asserts=False,
        num_devices=N_CORES,
    )

    x_d = nc.dram_tensor("x", [n_batches, S, D], F32, kind="ExternalInput")
    ln1_g_d = nc.dram_tensor("ln1_g", [D], F32, kind="ExternalInput")
    ln1_b_d = nc.dram_tensor("ln1_b", [D], F32, kind="ExternalInput")
    ln2_g_d = nc.dram_tensor("ln2_g", [D], F32, kind="ExternalInput")
    ln2_b_d = nc.dram_tensor("ln2_b", [D], F32, kind="ExternalInput")
    w1_d = nc.dram_tensor("W1", [D, FF], F32, kind="ExternalInput")
    b1_d = nc.dram_tensor("b1", [FF], F32, kind="ExternalInput")
    w2_d = nc.dram_tensor("W2", [FF, D], F32, kind="ExternalInput")
    b2_d = nc.dram_tensor("b2", [D], F32, kind="ExternalInput")
    out_d = nc.dram_tensor("out", [n_batches, S, D], F32, kind="ExternalOutput")
    if dbg:
        dbg_h = nc.dram_tensor("dbg_h", [C, P, D], F32, kind="ExternalOutput")
        dbg_ht = nc.dram_tensor("dbg_ht", [DC, 128, CW], F32, kind="ExternalOutput")
        dbg_at = nc.dram_tensor("dbg_at", [H, 65, CW], F32, kind="ExternalOutput")
        dbg_hn = nc.dram_tensor("dbg_hn", [C, P, D], F32, kind="ExternalOutput")
        dbg_h2 = nc.dram_tensor("dbg_h2", [C, P, D], F32, kind="ExternalOutput")
        dbg_r1 = nc.dram_tensor("dbg_r1", [FC, 128, CW], F32, kind="ExternalOutput")

    # Compile-time constants embedded in the NEFF.
    import ml_dtypes

    mask01_np = np.triu(np.ones((P, P), dtype=np.float32)).astype(ml_dtypes.bfloat16)
    ident_np = np.eye(128, dtype=np.float32)
    ones_np = np.ones((P, 1), dtype=ml_dtypes.bfloat16)
    mask01_d = nc.inline_tensor(mask01_np, name="mask01")
    ident_d = nc.inline_tensor(ident_np, name="ident128")
    ones_d = nc.inline_tensor(ones_np, name="onescol")

    def bcast(ap_1d, parts):
        return bass.AP(
            tensor=ap_1d.tensor, offset=ap_1d.offset, ap=[[0, parts], *ap_1d.ap]
        )

    with tile.TileContext(nc) as tc:
        with (
            tc.tile_pool(name="consts", bufs=1) as consts,
            tc.tile_pool(name="xin", bufs=2) as xin,
            tc.tile_pool(name="hpool", bufs=2) as hpool,
            tc.tile_pool(name="hbf", bufs=2) as hbf,
            tc.tile_pool(name="ht", bufs=8) as htp,
            tc.tile_pool(name="epool", bufs=8) as epool,
            tc.tile_pool(name="attn", bufs=2) as attnp,
            tc.tile_pool(name="small", bufs=8) as small,
            tc.tile_pool(name="relu1", bufs=FC) as relup,
            tc.tile_pool(name="outp", bufs=2) as outp,
            tc.tile_pool(name="psum", bufs=1, space="PSUM") as psum,
        ):
            # ---- one-time constant loads ----
            w1_sb = consts.tile([128, DC, FF], BF16)   # W1[128*dc+p, f]
            nc.gpsimd.dma_start(
                out=w1_sb, in_=w1_d.ap().rearrange("(c p) f -> p c f", p=128)
            )
            w2_sb = consts.tile([128, FC, D], BF16)    # W2[128*fc+p, d]
            nc.gpsimd.dma_start(
                out=w2_sb, in_=w2_d.ap().rearrange("(c p) f -> p c f", p=128)
            )
            b1_sb = consts.tile([128, FC], F32)        # b1[128*fc+p]
            nc.sync.dma_start(
                out=b1_sb, in_=b1_d.ap().rearrange("(c p) -> p c", p=128)
            )
            g1_sb = consts.tile([128, D], F32)
            nc.sync.dma_start(out=g1_sb, in_=bcast(ln1_g_d.ap(), 128))
            bb1_sb = consts.tile([128, D], F32)
            nc.sync.dma_start(out=bb1_sb, in_=bcast(ln1_b_d.ap(), 128))
            g2_sb = consts.tile([128, D], F32)
            nc.sync.dma_start(out=g2_sb, in_=bcast(ln2_g_d.ap(), 128))
            bb2_sb = consts.tile([128, D], F32)
            nc.sync.dma_start(out=bb2_sb, in_=bcast(ln2_b_d.ap(), 128))
            b2bc_sb = consts.tile([128, D], F32)
            nc.sync.dma_start(out=b2bc_sb, in_=bcast(b2_d.ap(), 128))
            mask_sb = consts.tile([P, P], BF16)
            nc.sync.dma_start(out=mask_sb, in_=mask01_d.ap())
            ident_sb = consts.tile([128, 128], F32)
            nc.sync.dma_start(out=ident_sb, in_=ident_d.ap())
            ones_sb = consts.tile([P, 1], BF16)
            nc.sync.dma_start(out=ones_sb, in_=ones_d.ap())
            eps_sb = consts.tile([128, 1], F32)
            nc.vector.memset(eps_sb, EPS)

            def layernorm(x_src_chunk, g_t, b_t, out_f32_chunk):
                """x_src_chunk: [P, D] fp32 slice; writes normalized fp32."""
                stats = small.tile([P, 6], F32, tag="stats")
                nc.vector.bn_stats(out=stats, in_=x_src_chunk)
                mv = small.tile([P, 2], F32, tag="mv")
                nc.vector.bn_aggr(out=mv, in_=stats)
                std = small.tile([P, 1], F32, tag="std")
                nc.scalar.activation(
                    out=std, in_=mv[:, 1:2], func=AF.Sqrt, bias=eps_sb[:P], scale=1.0
                )
                rstd = small.tile([P, 1], F32, tag="rstd")
                nc.vector.reciprocal(out=rstd, in_=std)
                nc.vector.tensor_scalar(
                    out=out_f32_chunk,
                    in0=x_src_chunk,
                    scalar1=mv[:, 0:1],
                    scalar2=rstd,
                    op0=OP.subtract,
                    op1=OP.mult,
                )
                # gain/bias (faithful application; GpSimd is otherwise idle)
                nc.gpsimd.tensor_mul(out_f32_chunk, out_f32_chunk, g_t[:P])
                nc.gpsimd.tensor_add(out_f32_chunk, out_f32_chunk, b_t[:P])

            def transpose_to(src_bf, dst_tiles):
                """src_bf [128, C, D] bf16 -> dst_tiles[dc] [128, C, 128]."""
                for dc in range(DC):
                    for c in range(C):
                        nc.sync.dma_start(
                            out=dst_tiles[dc][:, c, :],
                            in_=src_bf[:, c, 128 * dc : 128 * (dc + 1)],
                            transpose=True,
                        )

            for b in range(n_batches):
                # ---- load x[b] ----
                x_t = xin.tile([P, C, D], F32, tag="x")
                nc.sync.dma_start(
                    out=x_t, in_=x_d.ap()[b].rearrange("(c p) d -> p c d", p=P)
                )

                # ---- LN1 -> h (fp32) + bf16 copy ----
                h_f = hpool.tile([P, C, D], F32, tag="h")
                h_b = hbf.tile([128, C, D], BF16, tag="hb")
                # zero rows [96,128) first so the 3 pad rows (125-127) are
                # defined zeros for the DMA transpose; casts overwrite 96-124.
                nc.gpsimd.memset(h_b[96:128, :, :], 0.0)
                for c in range(C):
                    layernorm(x_t[:, c, :], g1_sb, bb1_sb, h_f[:, c, :])
                    nc.scalar.copy(out=h_b[:P, c, :], in_=h_f[:, c, :])

                # ---- h^T via DMA xbar (chunk-padded token axis) ----
                hT = [htp.tile([128, C, 128], BF16, tag="ht", name=f"hT{b}_{i}")
                      for i in range(DC)]
                transpose_to(h_b, hT)

                if dbg and b == 0:
                    nc.sync.dma_start(
                        out=dbg_h.ap().rearrange("c p d -> p c d"), in_=h_f
                    )
                    for dc in range(DC):
                        htf = hpool.tile([128, C, 128], F32, tag="dbght",
                                         name=f"dbght{dc}")
                        nc.scalar.copy(out=htf, in_=hT[dc])
                        nc.sync.dma_start(
                            out=dbg_ht.ap()[dc].rearrange("p (c q) -> p c q", q=128),
                            in_=htf,
                        )

                # ---- attention ----
                h_new = hpool.tile([P, C, D], F32, tag="hn")
                for j in range(DC):  # head pair (2j, 2j+1) lives in hT[j]
                    e_tiles = {}
                    for hh in (0, 1):  # row-packed: hh=0 rows 0-63, hh=1 rows 64-127
                        lo, hi = 64 * hh, 64 * (hh + 1)
                        for kb in range(C):
                            ps = psum.tile([P, C, 128], F32, tag="s", bufs=2,
                                           name=f"ps_{b}_{j}_{hh}_{kb}")
                            nc.tensor.matmul(
                                ps[:, kb:C, :],
                                lhsT=hT[j][lo:hi, kb, 0:P],
                                rhs=hT[j][lo:hi, kb:C, :],
                                start=True,
                                stop=True,
                            )
                            e_t = epool.tile([P, C, 128], BF16, tag="e",
                                             name=f"e_{b}_{j}_{hh}_{kb}")
                            nc.scalar.activation(
                                out=e_t[:, kb:C, :], in_=ps[:, kb:C, :],
                                func=AF.Exp, scale=0.125,
                            )
                            nc.vector.tensor_mul(
                                e_t[:, kb, 0:P], e_t[:, kb, 0:P], mask_sb
                            )
                            e_tiles[(hh, kb)] = e_t

                    for hh in (0, 1):
                        head = 2 * j + hh
                        pa = psum.tile([64, C, 128], F32, tag="att", bufs=2,
                                       name=f"pa_{b}_{head}")
                        pz = psum.tile([65, C, 128], F32, tag="tr", bufs=2,
                                       name=f"pz_{b}_{head}")
                        for kb in range(C):
                            e_t = e_tiles[(hh, kb)]
                            nc.tensor.matmul(
                                pa[0:64, kb:C, :],
                                lhsT=h_b[:P, kb, HD * head : HD * (head + 1)],
                                rhs=e_t[:, kb:C, :],
                                start=(kb == 0),
                                stop=(kb == C - 1),
                            )
                            nc.tensor.matmul(
                                pz[64:65, kb:C, :],
                                lhsT=ones_sb,
                                rhs=e_t[:, kb:C, :],
                                start=(kb == 0),
                                stop=(kb == C - 1),
                                tile_position=(0, 64),
                            )
                        at_sb = attnp.tile([65, C, 128], F32, tag="at",
                                           name=f"at_{b}_{head}")
                        nc.scalar.copy(out=at_sb[0:64], in_=pa[0:64])
                        nc.scalar.copy(out=at_sb[64:65], in_=pz[64:65])
                        if dbg and b == 0:
                            nc.sync.dma_start(
                                out=dbg_at.ap()[head].rearrange(
                                    "p (c q) -> p c q", q=128
                                ),
                                in_=at_sb,
                            )
                        for qc in range(C):
                            pt = psum.tile([P, 65], F32, tag="tr", bufs=2,
                                           padded_shape=[P, 512],
                                           name=f"pt_{b}_{head}_{qc}")
                            nc.tensor.transpose(
                                pt,
                                in_=at_sb[:, qc, 0:P],
                                identity=ident_sb[0:65, 0:65],
                            )
                            rz = small.tile([P, 1], F32, tag="rz")
                            nc.vector.reciprocal(out=rz, in_=pt[:, 64:65])
                            # h_new = attn^T * (1/Z) + h   (residual)
                            hn = h_new[:, qc, HD * head : HD * (head + 1)]
                            nc.vector.scalar_tensor_tensor(
                                out=hn,
                                in0=pt[:, 0:64],
                                scalar=rz,
                                in1=h_f[:, qc, HD * head : HD * (head + 1)],
                                op0=OP.mult,
                                op1=OP.add,
                            )
                    del e_tiles

                if dbg and b == 0:
                    nc.sync.dma_start(
                        out=dbg_hn.ap().rearrange("c p d -> p c d"), in_=h_new
                    )

                # ---- LN2 -> h2 (fp32) + bf16 copy ----
                h2_f = hpool.tile([P, C, D], F32, tag="h2")
                h2_b = hbf.tile([128, C, D], BF16, tag="h2b")
                nc.gpsimd.memset(h2_b[96:128, :, :], 0.0)
                for c in range(C):
                    layernorm(h_new[:, c, :], g2_sb, bb2_sb, h2_f[:, c, :])
                    nc.scalar.copy(out=h2_b[:P, c, :], in_=h2_f[:, c, :])
                if dbg and b == 0:
                    nc.sync.dma_start(
                        out=dbg_h2.ap().rearrange("c p d -> p c d"), in_=h2_f
                    )
                # fold b2 into the residual carrier: h2 <- h2 + b2
                for c in range(C):
                    nc.gpsimd.tensor_add(h2_f[:, c, :], h2_f[:, c, :], b2bc_sb[:P])

                # ---- h2^T via DMA xbar ----
                h2T = [htp.tile([128, C, 128], BF16, tag="h2t", name=f"h2T{b}_{i}")
                       for i in range(DC)]
                transpose_to(h2_b, h2T)

                # ---- FFN ----
                relu1 = [relup.tile([128, C, 128], BF16, tag="r1",
                                    name=f"relu1_{b}_{i}") for i in range(FC)]
                for fc in range(FC):
                    pf = psum.tile([128, C, 128], F32, tag="ffn", bufs=2,
                                   name=f"ff1_{b}_{fc}")
                    for dc in range(DC):
                        nc.tensor.matmul(
                            pf,
                            lhsT=w1_sb[:, dc, 128 * fc : 128 * (fc + 1)],
                            rhs=h2T[dc][:, :, :],
                            start=(dc == 0),
                            stop=(dc == DC - 1),
                        )
                    nc.scalar.activation(
                        out=relu1[fc], in_=pf, func=AF.Relu,
                        bias=b1_sb[:, fc : fc + 1], scale=1.0,
                    )
                if dbg and b == 0:
                    for fc in range(FC):
                        r1f = hpool.tile([128, C, 128], F32, tag="dbgr1",
                                         name=f"dbgr1_{fc}")
                        nc.scalar.copy(out=r1f, in_=relu1[fc])
                        nc.sync.dma_start(
                            out=dbg_r1.ap()[fc].rearrange("p (c q) -> p c q", q=128),
                            in_=r1f,
                        )

                out_t = outp.tile([P, C, D], F32, tag="o")
                for qc in range(C):
                    pf2 = psum.tile([P, D], F32, tag="ffn", bufs=2,
                                    name=f"ff2_{b}_{qc}")
                    for fc in range(FC):
                        nc.tensor.matmul(
                            pf2,
                            lhsT=relu1[fc][:, qc, 0:P],
                            rhs=w2_sb[:, fc, :],
                            start=(fc == 0),
                            stop=(fc == FC - 1),
                        )
                    nc.vector.scalar_tensor_tensor(
                        out=out_t[:, qc, :],
                        in0=pf2,
                        scalar=0.0,
                        in1=h2_f[:, qc, :],
                        op0=OP.bypass,
                        op1=OP.add,
                    )
                nc.sync.dma_start(
                    out=out_d.ap()[b].rearrange("(c p) d -> p c d", p=P), in_=out_t
                )

    nc.compile()
    return nc


_NC_CACHE = {}


def kernel(**inputs):
    n_batches = BPC
    key = n_batches
    if key not in _NC_CACHE:
        _NC_CACHE[key] = build(n_batches)
    nc = _NC_CACHE[key]

    x = np.ascontiguousarray(inputs["x"], dtype=np.float32)
    shared = {
        k: np.ascontiguousarray(inputs[k], dtype=np.float32)
        for k in ("ln1_g", "ln1_b", "ln2_g", "ln2_b", "W1", "b1", "W2", "b2")
    }
    in_maps = []
    for i in range(N_CORES):
        m = {"x": x[i * BPC : (i + 1) * BPC]}
        m.update(shared)
        in_maps.append(m)

    res = run_bass_kernel_spmd(nc, in_maps, core_ids=list(range(N_CORES)))
    out = np.concatenate([r["out"] for r in res.results], axis=0)
    return out.astype(np.float32)


# revision 15
# speedup vs baseline: 1.1931x; 1.1931x over previous
"""Trainium2 Bass kernel for nn_EncoderLayer (pre-LN transformer encoder layer).

Reference computation (per batch element b):
    h  = LN1(x)
    h  = h + causal_attention(h)      # q=k=v=h, 8 heads, head dim 64
    h2 = LN2(h)
    out = h2 + relu(h2 @ W1 + b1) @ W2 + b2

Sharding: pure data-parallel over batch. B=64 is split 8 ways; each of the 8
NeuronCores runs the identical NEFF on its own 8-batch shard with the full
weights. No collectives.

Per-core layout strategy:
  - Activations live in [tokens(partition), D(free)] layout; LayerNorm reduces
    along the free dim with bn_stats/bn_aggr.
  - Matmuls contract along the partition dim, so attention scores and the FFN
    need a [D, tokens] transposed copy of the activations; that transpose is
    done with the DMA xbar (bf16, zero compute-engine cost).  The xbar writes
    256B-aligned output tiles, so the token axis of every transposed tensor is
    chunk-padded: [*, 4, 128] with tokens (c, j), j < 125 valid and the three
    pad columns zero (h_b rows 125-127 are memset).  Downstream matmul APs
    either span whole chunks (moving operands; pad columns produce garbage in
    pad output columns that nothing reads) or slice [0:125] (stationary
    operands).
  - Attention computes S^T = [keys, queries] so the exp (ScalarE) can read
    straight out of PSUM with the 1/sqrt(hd) scale folded in; causality is
    handled by restricting every matmul/exp to query chunks >= the key block
    plus one 125x125 upper-triangular mask multiply on the diagonal block.
    No max-subtraction is needed: post-LN logits are bounded (|s|/8 <~ 20) and
    the HW exp LUT is accurate there (probed 1.1e-5 rel err).
  - AV uses [v | ones] as a 65-col stationary operand (built with one strided
    bf16 copy per key block) and E^T as the moving operand, so the softmax
    denominators Z land at output partition 64 of the same matmul.  A small PE
    transpose brings the attention output back to [tokens, D], where one
    scalar_tensor_tensor fuses the 1/Z normalize + residual add.
  - FFN: ff1^T = W1^T @ h2^T (W1 is naturally [D, dff] = the stationary
    layout), relu(.+b1) fused on ScalarE, then ff2 uses relu1^T as stationary
    to come back to [tokens, D] directly; residual + b2 fused on the way out.
  - All matmuls are bf16 (4x the fp32 rate on the PE); all accumulation and
    the residual spine stay fp32.
  - The batch loop is software-pipelined: stage A(b) = LN1 + h^T + attention,
    stage B(b) = LN2 + h2^T + FFN, emitted A(0) A(1) B(0) A(2) B(1) ... so the
    PE always has the next batch's attention matmuls queued while the serial
    LN2 -> cast -> transpose chain of the current batch runs on DVE/ACT/DMA.
    Without this the PE idles ~28us per batch and the HAM re-throttles to
    1.2 GHz every batch.
"""

import numpy as np

import concourse.bass as bass
import concourse.mybir as mybir
import concourse.tile as tile
from concourse import bacc
from concourse.bass_utils import run_bass_kernel_spmd

F32 = mybir.dt.float32
BF16 = mybir.dt.bfloat16
AF = mybir.ActivationFunctionType
OP = mybir.AluOpType

N_CORES = 8
B, S, D = 64, 500, 512
H, HD = 8, 64
FF = 2048
EPS = 1e-6

BPC = B // N_CORES  # batches per core
P = 125             # tokens per chunk
C = S // P          # 4 chunks per batch
CW = C * 128        # chunk-padded token width (512)
DC = D // 128       # 4 chunks of D
FC = FF // 128      # 16 chunks of dff


def build(n_batches=BPC, dbg=False):
    nc = bacc.Bacc(
        "TRN2",
        target_bir_lowering=False,
        debug=False,
        enable_asserts=False,
        num_devices=N_CORES,
    )

    x_d = nc.dram_tensor("x", [n_batches, S, D], F32, kind="ExternalInput")
    ln1_g_d = nc.dram_tensor("ln1_g", [D], F32, kind="ExternalInput")
    ln1_b_d = nc.dram_tensor("ln1_b", [D], F32, kind="ExternalInput")
    ln2_g_d = nc.dram_tensor("ln2_g", [D], F32, kind="ExternalInput")
    ln2_b_d = nc.dram_tensor("ln2_b", [D], F32, kind="ExternalInput")
    w1_d = nc.dram_tensor("W1", [D, FF], F32, kind="ExternalInput")
    b1_d = nc.dram_tensor("b1", [FF], F32, kind="ExternalInput")
    w2_d = nc.dram_tensor("W2", [FF, D], F32, kind="ExternalInput")
    b2_d = nc.dram_tensor("b2", [D], F32, kind="ExternalInput")
    out_d = nc.dram_tensor("out", [n_batches, S, D], F32, kind="ExternalOutput")
    if dbg:
        dbg_h = nc.dram_tensor("dbg_h", [C, P, D], F32, kind="ExternalOutput")
        dbg_ht = nc.dram_tensor("dbg_ht", [DC, 128, CW], F32, kind="ExternalOutput")
        dbg_at = nc.dram_tensor("dbg_at", [H, 65, CW], F32, kind="ExternalOutput")
        dbg_hn = nc.dram_tensor("dbg_hn", [C, P, D], F32, kind="ExternalOutput")
        dbg_h2 = nc.dram_tensor("dbg_h2", [C, P, D], F32, kind="ExternalOutput")
        dbg_r1 = nc.dram_tensor("dbg_r1", [FC, 128, CW], F32, kind="ExternalOutput")

    # Compile-time constants embedded in the NEFF.
    import ml_dtypes

    mask01_np = np.triu(np.ones((P, P), dtype=np.float32)).astype(ml_dtypes.bfloat16)
    ident_np = np.eye(128, dtype=np.float32)
    mask01_d = nc.inline_tensor(mask01_np, name="mask01")
    ident_d = nc.inline_tensor(ident_np, name="ident128")

    def bcast(ap_1d, parts):
        return bass.AP(
            tensor=ap_1d.tensor, offset=ap_1d.offset, ap=[[0, parts], *ap_1d.ap]
        )

    with tile.TileContext(nc) as tc:
        with (
            tc.tile_pool(name="consts", bufs=1) as consts,
            tc.tile_pool(name="xin", bufs=2) as xin,
            tc.tile_pool(name="hpool", bufs=2) as hpool,
            tc.tile_pool(name="hbf", bufs=2) as hbf,
            tc.tile_pool(name="ht", bufs=8) as htp,
            tc.tile_pool(name="epool", bufs=8) as epool,
            tc.tile_pool(name="attn", bufs=2) as attnp,
            tc.tile_pool(name="small", bufs=8) as small,
            tc.tile_pool(name="relu1", bufs=FC) as relup,
            tc.tile_pool(name="outp", bufs=2) as outp,
            tc.tile_pool(name="psum", bufs=1, space="PSUM") as psum,
        ):
            # ---- one-time constant loads ----
            w1_sb = consts.tile([128, DC, FF], BF16)   # W1[128*dc+p, f]
            nc.gpsimd.dma_start(
                out=w1_sb, in_=w1_d.ap().rearrange("(c p) f -> p c f", p=128)
            )
            w2_sb = consts.tile([128, FC, D], BF16)    # W2[128*fc+p, d]
            nc.gpsimd.dma_start(
                out=w2_sb, in_=w2_d.ap().rearrange("(c p) f -> p c f", p=128)
            )
            b1_sb = consts.tile([128, FC], F32)        # b1[128*fc+p]
            nc.sync.dma_start(
                out=b1_sb, in_=b1_d.ap().rearrange("(c p) -> p c", p=128)
            )
            g1_sb = consts.tile([128, D], F32)
            nc.sync.dma_start(out=g1_sb, in_=bcast(ln1_g_d.ap(), 128))
            bb1_sb = consts.tile([128, D], F32)
            nc.sync.dma_start(out=bb1_sb, in_=bcast(ln1_b_d.ap(), 128))
            g2_sb = consts.tile([128, D], F32)
            nc.sync.dma_start(out=g2_sb, in_=bcast(ln2_g_d.ap(), 128))
            bb2_sb = consts.tile([128, D], F32)
            nc.sync.dma_start(out=bb2_sb, in_=bcast(ln2_b_d.ap(), 128))
            b2bc_sb = consts.tile([128, D], F32)
            nc.sync.dma_start(out=b2bc_sb, in_=bcast(b2_d.ap(), 128))
            mask_sb = consts.tile([P, P], BF16)
            nc.sync.dma_start(out=mask_sb, in_=mask01_d.ap())
            ident_sb = consts.tile([128, 128], F32)
            nc.sync.dma_start(out=ident_sb, in_=ident_d.ap())
            eps_sb = consts.tile([128, 1], F32)
            nc.vector.memset(eps_sb, EPS)

            def layernorm(x_src_chunk, g_t, b_t, out_f32_chunk):
                """x_src_chunk: [P, D] fp32 slice; writes (x-mu)*rstd*g + b."""
                stats = small.tile([P, 6], F32, tag="stats")
                nc.vector.bn_stats(out=stats, in_=x_src_chunk)
                mv = small.tile([P, 2], F32, tag="mv")
                nc.vector.bn_aggr(out=mv, in_=stats)
                std = small.tile([P, 1], F32, tag="std")
                nc.scalar.activation(
                    out=std, in_=mv[:, 1:2], func=AF.Sqrt, bias=eps_sb[:P], scale=1.0
                )
                rstd = small.tile([P, 1], F32, tag="rstd")
                nc.vector.reciprocal(out=rstd, in_=std)
                nc.vector.tensor_scalar(
                    out=out_f32_chunk,
                    in0=x_src_chunk,
                    scalar1=mv[:, 0:1],
                    scalar2=rstd,
                    op0=OP.subtract,
                    op1=OP.mult,
                )
                # gain/bias (faithful application, cheap on DVE)
                nc.vector.tensor_mul(out_f32_chunk, out_f32_chunk, g_t[:P])
                nc.vector.tensor_add(out_f32_chunk, out_f32_chunk, b_t[:P])

            def transpose_to(src_bf, dst_tiles, eng):
                """src_bf [128, C, D] bf16 -> dst_tiles[dc] [128, C, 128]."""
                for dc in range(DC):
                    for c in range(C):
                        eng.dma_start(
                            out=dst_tiles[dc][:, c, :],
                            in_=src_bf[:, c, 128 * dc : 128 * (dc + 1)],
                            transpose=True,
                        )

            def stage_a(b):
                """LN1 + h^T + attention -> returns (h_f, h_new)."""
                x_t = xin.tile([P, C, D], F32, tag="x", name=f"x_{b}")
                nc.sync.dma_start(
                    out=x_t, in_=x_d.ap()[b].rearrange("(c p) d -> p c d", p=P)
                )

                h_f = hpool.tile([P, C, D], F32, tag="h", name=f"h_{b}")
                h_b = hbf.tile([128, C, D], BF16, tag="hb", name=f"hb_{b}")
                # zero rows [96,128) first so the 3 pad rows (125-127) are
                # defined zeros for the DMA transpose; casts overwrite 96-124.
                nc.gpsimd.memset(h_b[96:128, :, :], 0.0)
                for c in range(C):
                    layernorm(x_t[:, c, :], g1_sb, bb1_sb, h_f[:, c, :])
                    nc.scalar.copy(out=h_b[:P, c, :], in_=h_f[:, c, :])

                hT = [htp.tile([128, C, 128], BF16, tag="ht", name=f"hT{b}_{i}")
                      for i in range(DC)]
                transpose_to(h_b, hT, nc.sync)

                # [v | ones] stationaries: vo[p, kb, h, 0:64] = h_b head slice,
                # vo[p, kb, h, 64] = 1.  One strided copy per key block.
                vo = hbf.tile([P, C, H, HD + 1], BF16, tag="vo", name=f"vo_{b}")
                nc.vector.memset(vo[:, :, :, HD : HD + 1], 1.0)
                for kb in range(C):
                    nc.vector.tensor_copy(
                        out=vo[:, kb, :, 0:HD],
                        in_=h_b[:P, kb, :].rearrange("p (h d) -> p h d", h=H),
                    )

                if dbg and b == 0:
                    nc.sync.dma_start(
                        out=dbg_h.ap().rearrange("c p d -> p c d"), in_=h_f
                    )
                    for dc in range(DC):
                        htf = hpool.tile([128, C, 128], F32, tag="dbght",
                                         name=f"dbght{dc}")
                        nc.scalar.copy(out=htf, in_=hT[dc])
                        nc.sync.dma_start(
                            out=dbg_ht.ap()[dc].rearrange("p (c q) -> p c q", q=128),
                            in_=htf,
                        )

                h_new = hpool.tile([P, C, D], F32, tag="hn", name=f"hn_{b}")
                for j in range(DC):  # head pair (2j, 2j+1) lives in hT[j]
                    e_tiles = {}
                    for hh in (0, 1):  # row-packed: hh=0 rows 0-63, hh=1 rows 64-127
                        lo, hi = 64 * hh, 64 * (hh + 1)
                        for kb in range(C):
                            ps = psum.tile([P, C, 128], F32, tag="s", bufs=2,
                                           name=f"ps_{b}_{j}_{hh}_{kb}")
                            nc.tensor.matmul(
                                ps[:, kb:C, :],
                                lhsT=hT[j][lo:hi, kb, 0:P],
                                rhs=hT[j][lo:hi, kb:C, :],
                                start=True,
                                stop=True,
                            )
                            e_t = epool.tile([P, C, 128], BF16, tag="e",
                                             name=f"e_{b}_{j}_{hh}_{kb}")
                            nc.scalar.activation(
                                out=e_t[:, kb:C, :], in_=ps[:, kb:C, :],
                                func=AF.Exp, scale=0.125,
                            )
                            nc.vector.tensor_mul(
                                e_t[:, kb, 0:P], e_t[:, kb, 0:P], mask_sb
                            )
                            e_tiles[(hh, kb)] = e_t

                    for hh in (0, 1):
                        head = 2 * j + hh
                        pa = psum.tile([65, C, 128], F32, tag="att", bufs=2,
                                       name=f"pa_{b}_{head}")
                        for kb in range(C):
                            e_t = e_tiles[(hh, kb)]
                            nc.tensor.matmul(
                                pa[0:65, kb:C, :],
                                lhsT=vo[:, kb, head, :],
                                rhs=e_t[:, kb:C, :],
                                start=(kb == 0),
                                stop=(kb == C - 1),
                            )
                        at_sb = attnp.tile([65, C, 128], F32, tag="at",
                                           name=f"at_{b}_{head}")
                        nc.scalar.copy(out=at_sb, in_=pa)
                        if dbg and b == 0:
                            nc.sync.dma_start(
                                out=dbg_at.ap()[head].rearrange(
                                    "p (c q) -> p c q", q=128
                                ),
                                in_=at_sb,
                            )
                        for qc in range(C):
                            pt = psum.tile([P, 65], F32, tag="tr", bufs=2,
                                           padded_shape=[P, 512],
                                           name=f"pt_{b}_{head}_{qc}")
                            nc.tensor.transpose(
                                pt,
                                in_=at_sb[:, qc, 0:P],
                                identity=ident_sb[0:65, 0:65],
                            )
                            rz = small.tile([P, 1], F32, tag="rz")
                            nc.vector.reciprocal(out=rz, in_=pt[:, 64:65])
                            # h_new = attn^T * (1/Z) + h   (residual)
                            hn = h_new[:, qc, HD * head : HD * (head + 1)]
                            nc.vector.scalar_tensor_tensor(
                                out=hn,
                                in0=pt[:, 0:64],
                                scalar=rz,
                                in1=h_f[:, qc, HD * head : HD * (head + 1)],
                                op0=OP.mult,
                                op1=OP.add,
                            )
                    del e_tiles

                if dbg and b == 0:
                    nc.sync.dma_start(
                        out=dbg_hn.ap().rearrange("c p d -> p c d"), in_=h_new
                    )
                return h_new

            def stage_b(b, h_new):
                """LN2 + h2^T + FFN + output DMA."""
                h2_f = hpool.tile([P, C, D], F32, tag="h2", name=f"h2_{b}")
                h2_b = hbf.tile([128, C, D], BF16, tag="h2b", name=f"h2b_{b}")
                nc.gpsimd.memset(h2_b[96:128, :, :], 0.0)
                for c in range(C):
                    layernorm(h_new[:, c, :], g2_sb, bb2_sb, h2_f[:, c, :])
                    nc.scalar.copy(out=h2_b[:P, c, :], in_=h2_f[:, c, :])
                if dbg and b == 0:
                    nc.sync.dma_start(
                        out=dbg_h2.ap().rearrange("c p d -> p c d"), in_=h2_f
                    )
                # fold b2 into the residual carrier: h2 <- h2 + b2
                for c in range(C):
                    nc.vector.tensor_add(h2_f[:, c, :], h2_f[:, c, :], b2bc_sb[:P])

                h2T = [htp.tile([128, C, 128], BF16, tag="h2t", name=f"h2T{b}_{i}")
                       for i in range(DC)]
                transpose_to(h2_b, h2T, nc.scalar)

                relu1 = [relup.tile([128, C, 128], BF16, tag="r1",
                                    name=f"relu1_{b}_{i}") for i in range(FC)]
                for fc in range(FC):
                    pf = psum.tile([128, C, 128], F32, tag="ffn", bufs=2,
                                   name=f"ff1_{b}_{fc}")
                    for dc in range(DC):
                        nc.tensor.matmul(
                            pf,
                            lhsT=w1_sb[:, dc, 128 * fc : 128 * (fc + 1)],
                            rhs=h2T[dc][:, :, :],
                            start=(dc == 0),
                            stop=(dc == DC - 1),
                        )
                    nc.scalar.activation(
                        out=relu1[fc], in_=pf, func=AF.Relu,
                        bias=b1_sb[:, fc : fc + 1], scale=1.0,
                    )
                if dbg and b == 0:
                    for fc in range(FC):
                        r1f = hpool.tile([128, C, 128], F32, tag="dbgr1",
                                         name=f"dbgr1_{fc}")
                        nc.scalar.copy(out=r1f, in_=relu1[fc])
                        nc.sync.dma_start(
                            out=dbg_r1.ap()[fc].rearrange("p (c q) -> p c q", q=128),
                            in_=r1f,
                        )

                out_t = outp.tile([P, C, D], F32, tag="o", name=f"o_{b}")
                for qc in range(C):
                    pf2 = psum.tile([P, D], F32, tag="ffn", bufs=2,
                                    name=f"ff2_{b}_{qc}")
                    for fc in range(FC):
                        nc.tensor.matmul(
                            pf2,
                            lhsT=relu1[fc][:, qc, 0:P],
                            rhs=w2_sb[:, fc, :],
                            start=(fc == 0),
                            stop=(fc == FC - 1),
                        )
                    nc.vector.scalar_tensor_tensor(
                        out=out_t[:, qc, :],
                        in0=pf2,
                        scalar=0.0,
                        in1=h2_f[:, qc, :],
                        op0=OP.bypass,
                        op1=OP.add,
                    )
                nc.sync.dma_start(
                    out=out_d.ap()[b].rearrange("(c p) d -> p c d", p=P), in_=out_t
                )

            # software pipeline: A(0) A(1) B(0) A(2) B(1) ... B(n-1)
            hn_prev = None
            for b in range(n_batches):
                hn = stage_a(b)
                if hn_prev is not None:
                    stage_b(b - 1, hn_prev)
                hn_prev = hn
            stage_b(n_batches - 1, hn_prev)

    nc.compile()
    return nc


_NC_CACHE = {}


def kernel(**inputs):
    n_batches = BPC
    key = n_batches
    if key not in _NC_CACHE:
        _NC_CACHE[key] = build(n_batches)
    nc = _NC_CACHE[key]

    x = np.ascontiguousarray(inputs["x"], dtype=np.float32)
    shared = {
        k: np.ascontiguousarray(inputs[k], dtype=np.float32)
        for k in ("ln1_g", "ln1_b", "ln2_g", "ln2_b", "W1", "b1", "W2", "b2")
    }
    in_maps = []
    for i in range(N_CORES):
        m = {"x": x[i * BPC : (i + 1) * BPC]}
        m.update(shared)
        in_maps.append(m)

    res = run_bass_kernel_spmd(nc, in_maps, core_ids=list(range(N_CORES)))
    out = np.concatenate([r["out"] for r in res.results], axis=0)
    return out.astype(np.float32)


# revision 18
# speedup vs baseline: 1.3447x; 1.1271x over previous
"""Trainium2 Bass kernel for nn_EncoderLayer (pre-LN transformer encoder layer).

Reference computation (per batch element b):
    h  = LN1(x)
    h  = h + causal_attention(h)      # q=k=v=h, 8 heads, head dim 64
    h2 = LN2(h)
    out = h2 + relu(h2 @ W1 + b1) @ W2 + b2

Sharding: pure data-parallel over batch. B=64 is split 8 ways; each of the 8
NeuronCores runs the identical NEFF on its own 8-batch shard with the full
weights. No collectives.

Layout/engine strategy (see git-less history in comments):
  - Activations in [tokens(P), D(free)]; LN via bn_stats/bn_aggr (DVE) +
    Sqrt (ACT) + reciprocal (DVE); gain/bias on GpSimd (off critical engines).
  - Transposed copies ([D, tokens]) for matmul contraction via DMA-xbar
    transpose (bf16).  Xbar writes 256B tiles -> token axis is chunk-padded
    [*, 4, 128] (3 zero pad cols per 125-token chunk).  h^T on the Sync HWDGE
    queue, h2^T on the Scalar HWDGE queue.
  - Attention: S^T = [keys, queries] (2-head row-packed matmuls, K=64 each in
    array rows 0-63/64-127); exp on ACT straight from PSUM with scale=1/8
    folded in (no max-subtraction: post-LN logits bounded, HW exp LUT accurate
    to 1e-5 there); causality via triangle-restricted ranges + one 125x125
    mask multiply (DVE) on the diagonal block; AV with [v | ones] 65-col
    stationaries so the softmax denominator Z lands at out partition 64 of the
    same matmul; PSUM->SBUF copy as bf16 (DVE), PE-transpose back (bf16), one
    strided reciprocal per head, normalize+residual fused in one
    scalar_tensor_tensor per (head, chunk).
  - FFN: ff1^T = W1^T @ h2^T (W1's natural layout is the stationary layout),
    relu(.+b1) fused on ACT; ff2 uses relu1^T slices as stationaries to come
    back to [tokens, D]; residual (+b2, folded into h2 on GpSimd) fused in the
    epilogue STT (DVE).
  - All matmuls bf16 (4x fp32 PE rate); accumulation + residual spine fp32.
  - Emission is software-pipelined per cycle b:
        ln2(b-1), ln1(b+1), attn(b), ffn(b-1)
    so every serial LN -> cast -> transpose chain runs on DVE/GpSimd/ACT/DMA
    while the PE chews the previous stage's matmuls, and the PE never idles
    long enough for the HAM to re-throttle it to 1.2 GHz.  Inside attn the
    per-head epilogue (PSUM copy, transpose-back, STT) for head pair j-1 is
    emitted between S^T(j) and AV(j) for the same reason.
"""

import numpy as np

import concourse.bass as bass
import concourse.mybir as mybir
import concourse.tile as tile
from concourse import bacc
from concourse.bass_utils import run_bass_kernel_spmd

F32 = mybir.dt.float32
BF16 = mybir.dt.bfloat16
AF = mybir.ActivationFunctionType
OP = mybir.AluOpType

N_CORES = 8
B, S, D = 64, 500, 512
H, HD = 8, 64
FF = 2048
EPS = 1e-6

BPC = B // N_CORES  # batches per core
P = 125             # tokens per chunk
C = S // P          # 4 chunks per batch
CW = C * 128        # chunk-padded token width (512)
DC = D // 128       # 4 chunks of D
FC = FF // 128      # 16 chunks of dff


def build(n_batches=BPC, dbg=False):
    nc = bacc.Bacc(
        "TRN2",
        target_bir_lowering=False,
        debug=False,
        enable_asserts=False,
        num_devices=N_CORES,
    )

    x_d = nc.dram_tensor("x", [n_batches, S, D], F32, kind="ExternalInput")
    ln1_g_d = nc.dram_tensor("ln1_g", [D], F32, kind="ExternalInput")
    ln1_b_d = nc.dram_tensor("ln1_b", [D], F32, kind="ExternalInput")
    ln2_g_d = nc.dram_tensor("ln2_g", [D], F32, kind="ExternalInput")
    ln2_b_d = nc.dram_tensor("ln2_b", [D], F32, kind="ExternalInput")
    w1_d = nc.dram_tensor("W1", [D, FF], F32, kind="ExternalInput")
    b1_d = nc.dram_tensor("b1", [FF], F32, kind="ExternalInput")
    w2_d = nc.dram_tensor("W2", [FF, D], F32, kind="ExternalInput")
    b2_d = nc.dram_tensor("b2", [D], F32, kind="ExternalInput")
    out_d = nc.dram_tensor("out", [n_batches, S, D], F32, kind="ExternalOutput")
    if dbg:
        dbg_h = nc.dram_tensor("dbg_h", [C, P, D], F32, kind="ExternalOutput")
        dbg_ht = nc.dram_tensor("dbg_ht", [DC, 128, CW], F32, kind="ExternalOutput")
        dbg_at = nc.dram_tensor("dbg_at", [H, 65, CW], F32, kind="ExternalOutput")
        dbg_hn = nc.dram_tensor("dbg_hn", [C, P, D], F32, kind="ExternalOutput")
        dbg_h2 = nc.dram_tensor("dbg_h2", [C, P, D], F32, kind="ExternalOutput")
        dbg_r1 = nc.dram_tensor("dbg_r1", [FC, 128, CW], F32, kind="ExternalOutput")

    # Compile-time constants embedded in the NEFF.
    import ml_dtypes

    mask01_np = np.triu(np.ones((P, P), dtype=np.float32)).astype(ml_dtypes.bfloat16)
    ident_np = np.eye(128, dtype=np.float32).astype(ml_dtypes.bfloat16)
    mask01_d = nc.inline_tensor(mask01_np, name="mask01")
    ident_d = nc.inline_tensor(ident_np, name="identbf")

    def bcast(ap_1d, parts):
        return bass.AP(
            tensor=ap_1d.tensor, offset=ap_1d.offset, ap=[[0, parts], *ap_1d.ap]
        )

    with tile.TileContext(nc) as tc:
        with (
            tc.tile_pool(name="consts", bufs=1) as consts,
            tc.tile_pool(name="xin", bufs=2) as xin,
            tc.tile_pool(name="hpool", bufs=2) as hpool,
            tc.tile_pool(name="hbf", bufs=1) as hbf,
            tc.tile_pool(name="ht", bufs=8) as htp,
            tc.tile_pool(name="epool", bufs=12) as epool,
            tc.tile_pool(name="attn", bufs=2) as attnp,
            tc.tile_pool(name="small", bufs=8) as small,
            tc.tile_pool(name="relu1", bufs=FC) as relup,
            tc.tile_pool(name="outp", bufs=2) as outp,
            tc.tile_pool(name="psum", bufs=1, space="PSUM") as psum,
        ):
            # ---- one-time constant loads ----
            w1_sb = consts.tile([128, DC, FF], BF16)   # W1[128*dc+p, f]
            nc.gpsimd.dma_start(
                out=w1_sb, in_=w1_d.ap().rearrange("(c p) f -> p c f", p=128)
            )
            w2_sb = consts.tile([128, FC, D], BF16)    # W2[128*fc+p, d]
            nc.gpsimd.dma_start(
                out=w2_sb, in_=w2_d.ap().rearrange("(c p) f -> p c f", p=128)
            )
            b1_sb = consts.tile([128, FC], F32)        # b1[128*fc+p]
            nc.sync.dma_start(
                out=b1_sb, in_=b1_d.ap().rearrange("(c p) -> p c", p=128)
            )
            g1_sb = consts.tile([128, D], F32)
            nc.sync.dma_start(out=g1_sb, in_=bcast(ln1_g_d.ap(), 128))
            bb1_sb = consts.tile([128, D], F32)
            nc.sync.dma_start(out=bb1_sb, in_=bcast(ln1_b_d.ap(), 128))
            g2_sb = consts.tile([128, D], F32)
            nc.sync.dma_start(out=g2_sb, in_=bcast(ln2_g_d.ap(), 128))
            bb2_sb = consts.tile([128, D], F32)
            nc.sync.dma_start(out=bb2_sb, in_=bcast(ln2_b_d.ap(), 128))
            b2bc_sb = consts.tile([128, D], F32)
            nc.sync.dma_start(out=b2bc_sb, in_=bcast(b2_d.ap(), 128))
            mask_sb = consts.tile([P, P], BF16)
            nc.sync.dma_start(out=mask_sb, in_=mask01_d.ap())
            ident_sb = consts.tile([128, 128], BF16)
            nc.sync.dma_start(out=ident_sb, in_=ident_d.ap())
            eps_sb = consts.tile([128, 1], F32)
            nc.vector.memset(eps_sb, EPS)

            def layernorm(x_src_chunk, g_t, b_t, out_f32_chunk):
                """x_src_chunk: [P, D] fp32 slice; writes (x-mu)*rstd*g + b."""
                stats = small.tile([P, 6], F32, tag="stats")
                nc.vector.bn_stats(out=stats, in_=x_src_chunk)
                mv = small.tile([P, 2], F32, tag="mv")
                nc.vector.bn_aggr(out=mv, in_=stats)
                std = small.tile([P, 1], F32, tag="std")
                nc.scalar.activation(
                    out=std, in_=mv[:, 1:2], func=AF.Sqrt, bias=eps_sb[:P], scale=1.0
                )
                rstd = small.tile([P, 1], F32, tag="rstd")
                nc.vector.reciprocal(out=rstd, in_=std)
                nc.vector.tensor_scalar(
                    out=out_f32_chunk,
                    in0=x_src_chunk,
                    scalar1=mv[:, 0:1],
                    scalar2=rstd,
                    op0=OP.subtract,
                    op1=OP.mult,
                )
                # gain/bias (faithful application; GpSimd is off the hot path)
                nc.gpsimd.tensor_mul(out_f32_chunk, out_f32_chunk, g_t[:P])
                nc.gpsimd.tensor_add(out_f32_chunk, out_f32_chunk, b_t[:P])

            def transpose_to(src_bf, dst_tiles, eng):
                """src_bf [128, C, D] bf16 -> dst_tiles[dc] [128, C, 128]."""
                for dc in range(DC):
                    for c in range(C):
                        eng.dma_start(
                            out=dst_tiles[dc][:, c, :],
                            in_=src_bf[:, c, 128 * dc : 128 * (dc + 1)],
                            transpose=True,
                        )

            state = {}

            def stage_ln1(b):
                """x load + LN1 + bf16 cast + h^T + [v|1] stationaries."""
                x_t = xin.tile([P, C, D], F32, tag="x", name=f"x_{b}")
                nc.sync.dma_start(
                    out=x_t, in_=x_d.ap()[b].rearrange("(c p) d -> p c d", p=P)
                )
                h_f = hpool.tile([P, C, D], F32, tag="h", name=f"h_{b}")
                h_b = hbf.tile([128, C, D], BF16, tag="hb", name=f"hb_{b}")
                nc.gpsimd.memset(h_b[96:128, :, :], 0.0)
                for c in range(C):
                    layernorm(x_t[:, c, :], g1_sb, bb1_sb, h_f[:, c, :])
                    nc.vector.tensor_copy(out=h_b[:P, c, :], in_=h_f[:, c, :])
                hT = [htp.tile([128, C, 128], BF16, tag="ht", name=f"hT{b}_{i}")
                      for i in range(DC)]
                transpose_to(h_b, hT, nc.sync)
                vo = hbf.tile([P, C, H, HD + 1], BF16, tag="vo", bufs=2,
                              name=f"vo_{b}")
                nc.vector.memset(vo[:, :, :, HD : HD + 1], 1.0)
                for kb in range(C):
                    nc.vector.tensor_copy(
                        out=vo[:, kb, :, 0:HD],
                        in_=h_b[:P, kb, :].rearrange("p (h d) -> p h d", h=H),
                    )
                if dbg and b == 0:
                    nc.sync.dma_start(
                        out=dbg_h.ap().rearrange("c p d -> p c d"), in_=h_f
                    )
                    for dc in range(DC):
                        htf = hpool.tile([128, C, 128], F32, tag="dbght",
                                         name=f"dbght{dc}")
                        nc.scalar.copy(out=htf, in_=hT[dc])
                        nc.sync.dma_start(
                            out=dbg_ht.ap()[dc].rearrange("p (c q) -> p c q", q=128),
                            in_=htf,
                        )
                state[b] = {"h_f": h_f, "hT": hT, "vo": vo}

            def attn_epilogue(b, head, pa, h_f, h_new):
                """PSUM attn^T -> bf16 SBUF -> PE transpose -> 1/Z -> residual."""
                at_sb = attnp.tile([65, C, 128], BF16, tag="at",
                                   name=f"at_{b}_{head}")
                nc.vector.tensor_copy(out=at_sb, in_=pa)
                if dbg and b == 0:
                    atf = hpool.tile([65, C, 128], F32, tag="dbgat",
                                     name=f"dbgat_{head}")
                    nc.scalar.copy(out=atf, in_=at_sb)
                    nc.sync.dma_start(
                        out=dbg_at.ap()[head].rearrange("p (c q) -> p c q", q=128),
                        in_=atf,
                    )
                pt = psum.tile([P, C, 65], BF16, tag="tr", bufs=2,
                               padded_shape=[P, C, 256], name=f"pt_{b}_{head}")
                for qc in range(C):
                    nc.tensor.transpose(
                        pt[:, qc, :],
                        in_=at_sb[:, qc, 0:P],
                        identity=ident_sb[0:65, 0:65],
                    )
                rz = small.tile([P, C], F32, tag="rz", name=f"rz_{b}_{head}")
                nc.vector.reciprocal(out=rz, in_=pt[:, :, 64])
                for qc in range(C):
                    hn = h_new[:, qc, HD * head : HD * (head + 1)]
                    nc.vector.scalar_tensor_tensor(
                        out=hn,
                        in0=pt[:, qc, 0:64],
                        scalar=rz[:, qc : qc + 1],
                        in1=h_f[:, qc, HD * head : HD * (head + 1)],
                        op0=OP.mult,
                        op1=OP.add,
                    )

            def stage_attn(b):
                st = state[b]
                h_f, hT, vo = st["h_f"], st["hT"], st["vo"]
                h_new = hpool.tile([P, C, D], F32, tag="hn", name=f"hn_{b}")
                pending = []  # (head, pa) awaiting epilogue
                for j in range(DC):  # head pair (2j, 2j+1) lives in hT[j]
                    e_tiles = {}
                    for hh in (0, 1):  # row-packed: rows 0-63 / 64-127
                        lo, hi = 64 * hh, 64 * (hh + 1)
                        for kb in range(C):
                            ps = psum.tile([P, C, 128], F32, tag="s", bufs=2,
                                           name=f"ps_{b}_{j}_{hh}_{kb}")
                            nc.tensor.matmul(
                                ps[:, kb:C, :],
                                lhsT=hT[j][lo:hi, kb, 0:P],
                                rhs=hT[j][lo:hi, kb:C, :],
                                start=True,
                                stop=True,
                            )
                            e_t = epool.tile([P, C, 128], BF16, tag="e",
                                             name=f"e_{b}_{j}_{hh}_{kb}")
                            nc.scalar.activation(
                                out=e_t[:, kb:C, :], in_=ps[:, kb:C, :],
                                func=AF.Exp, scale=0.125,
                            )
                            nc.vector.tensor_mul(
                                e_t[:, kb, 0:P], e_t[:, kb, 0:P], mask_sb
                            )
                            e_tiles[(hh, kb)] = e_t
                    # epilogue of the previous pair runs while S^T(j) is on PE
                    for head, pa in pending:
                        attn_epilogue(b, head, pa, h_f, h_new)
                    pending = []
                    for hh in (0, 1):
                        head = 2 * j + hh
                        pa = psum.tile([65, C, 128], F32, tag="att", bufs=2,
                                       name=f"pa_{b}_{head}")
                        for kb in range(C):
                            nc.tensor.matmul(
                                pa[0:65, kb:C, :],
                                lhsT=vo[:, kb, head, :],
                                rhs=e_tiles[(hh, kb)][:, kb:C, :],
                                start=(kb == 0),
                                stop=(kb == C - 1),
                            )
                        pending.append((head, pa))
                for head, pa in pending:
                    attn_epilogue(b, head, pa, h_f, h_new)
                if dbg and b == 0:
                    nc.sync.dma_start(
                        out=dbg_hn.ap().rearrange("c p d -> p c d"), in_=h_new
                    )
                st["h_new"] = h_new

            def stage_ln2(b):
                st = state[b]
                h_new = st["h_new"]
                h2_f = hpool.tile([P, C, D], F32, tag="h2", name=f"h2_{b}")
                h2_b = hbf.tile([128, C, D], BF16, tag="h2b", name=f"h2b_{b}")
                nc.gpsimd.memset(h2_b[96:128, :, :], 0.0)
                for c in range(C):
                    layernorm(h_new[:, c, :], g2_sb, bb2_sb, h2_f[:, c, :])
                    nc.vector.tensor_copy(out=h2_b[:P, c, :], in_=h2_f[:, c, :])
                if dbg and b == 0:
                    nc.sync.dma_start(
                        out=dbg_h2.ap().rearrange("c p d -> p c d"), in_=h2_f
                    )
                # fold b2 into the residual carrier: h2 <- h2 + b2
                for c in range(C):
                    nc.gpsimd.tensor_add(h2_f[:, c, :], h2_f[:, c, :], b2bc_sb[:P])
                h2T = [htp.tile([128, C, 128], BF16, tag="h2t", name=f"h2T{b}_{i}")
                       for i in range(DC)]
                transpose_to(h2_b, h2T, nc.scalar)
                st["h2_f"] = h2_f
                st["h2T"] = h2T

            def stage_ffn(b):
                st = state.pop(b)
                h2_f, h2T = st["h2_f"], st["h2T"]
                relu1 = [relup.tile([128, C, 128], BF16, tag="r1",
                                    name=f"relu1_{b}_{i}") for i in range(FC)]
                for fc in range(FC):
                    pf = psum.tile([128, C, 128], F32, tag="ffn", bufs=2,
                                   name=f"ff1_{b}_{fc}")
                    for dc in range(DC):
                        nc.tensor.matmul(
                            pf,
                            lhsT=w1_sb[:, dc, 128 * fc : 128 * (fc + 1)],
                            rhs=h2T[dc][:, :, :],
                            start=(dc == 0),
                            stop=(dc == DC - 1),
                        )
                    nc.scalar.activation(
                        out=relu1[fc], in_=pf, func=AF.Relu,
                        bias=b1_sb[:, fc : fc + 1], scale=1.0,
                    )
                if dbg and b == 0:
                    for fc in range(FC):
                        r1f = hpool.tile([128, C, 128], F32, tag="dbgr1",
                                         name=f"dbgr1_{fc}")
                        nc.scalar.copy(out=r1f, in_=relu1[fc])
                        nc.sync.dma_start(
                            out=dbg_r1.ap()[fc].rearrange("p (c q) -> p c q", q=128),
                            in_=r1f,
                        )
                out_t = outp.tile([P, C, D], F32, tag="o", name=f"o_{b}")
                for qc in range(C):
                    pf2 = psum.tile([P, D], F32, tag="ffn", bufs=2,
                                    name=f"ff2_{b}_{qc}")
                    for fc in range(FC):
                        nc.tensor.matmul(
                            pf2,
                            lhsT=relu1[fc][:, qc, 0:P],
                            rhs=w2_sb[:, fc, :],
                            start=(fc == 0),
                            stop=(fc == FC - 1),
                        )
                    nc.vector.scalar_tensor_tensor(
                        out=out_t[:, qc, :],
                        in0=pf2,
                        scalar=0.0,
                        in1=h2_f[:, qc, :],
                        op0=OP.bypass,
                        op1=OP.add,
                    )
                nc.sync.dma_start(
                    out=out_d.ap()[b].rearrange("(c p) d -> p c d", p=P), in_=out_t
                )

            # software pipeline:
            # ln1(0) ln1(1) attn(0) | ln2(0) ln1(2) attn(1) ffn(0) | ...
            stage_ln1(0)
            if n_batches > 1:
                stage_ln1(1)
            stage_attn(0)
            for b in range(1, n_batches):
                stage_ln2(b - 1)
                if b + 1 < n_batches:
                    stage_ln1(b + 1)
                stage_attn(b)
                stage_ffn(b - 1)
            stage_ln2(n_batches - 1)
            stage_ffn(n_batches - 1)

    nc.compile()
    return nc


_NC_CACHE = {}


def kernel(**inputs):
    n_batches = BPC
    key = n_batches
    if key not in _NC_CACHE:
        _NC_CACHE[key] = build(n_batches)
    nc = _NC_CACHE[key]

    x = np.ascontiguousarray(inputs["x"], dtype=np.float32)
    shared = {
        k: np.ascontiguousarray(inputs[k], dtype=np.float32)
        for k in ("ln1_g", "ln1_b", "ln2_g", "ln2_b", "W1", "b1", "W2", "b2")
    }
    in_maps = []
    for i in range(N_CORES):
        m = {"x": x[i * BPC : (i + 1) * BPC]}
        m.update(shared)
        in_maps.append(m)

    res = run_bass_kernel_spmd(nc, in_maps, core_ids=list(range(N_CORES)))
    out = np.concatenate([r["out"] for r in res.results], axis=0)
    return out.astype(np.float32)
